# revision 19
# baseline (speedup 1.0000x reference)
"""GCN layer (gnn_message_passing) on 8 Trainium2 NeuronCores.

Reference computation:
    deg = segment_sum(ones, hs)              # in-degree of each node (rows hs)
    s   = deg ** -0.5
    agg[h] = sum over edges (h, t) of s[t] * feats[t]
    out = relu((s[:, None] * agg) @ W.T)

Distribution strategy (per the sharding hint):
  * Nodes are sharded across the 8 cores (12500 each); edges are partitioned
    by destination (hs) so the segment-sum is core-local. feats is replicated
    to every core's HBM.
  * y-precompute: each core scales the full table once, y[t] = s[t]*x[t], and
    stores it as fp16 in internal DRAM (4 chunks of 25000 rows so dma_gather's
    int16 indices reach every row). This folds the source normalization into
    the gathered data (no per-block scaling) and halves gather traffic.
  * Edges are bucketed by (dest group of 128, source chunk); each 128-edge
    block becomes one matmul: agg[f, d] += G_k[e, f]^T @ S_k[e, d], with G_k
    gathered fp16 y-rows and S_k an unscaled one-hot built by ONE broadcast
    DVE tensor_tensor per (superblock, chunk) - not per block.
  * The dest normalization s[h] is applied as the per-partition `scale` of the
    output Relu activation (relu(z*s) with s>0).
  * Gathers use batched multi-packet dma_gather (up to 8192 rows per call;
    single-packet mode crashes the SWDGE ring above ~1024 rows).
  * The program is SPMD (one BIR for all cores), so per-(group, chunk) block
    counts are baked as the max over cores.
  * Host-side preprocessing is integer-only bookkeeping (bucketing, padding,
    int16 index packing, integer degree tables). All floating-point math
    (rsqrt, scaling, SpMM, linear, relu) runs on device.
"""

import numpy as np

import concourse.bacc as bacc
import concourse.bass as bass
import concourse.mybir as mybir
import concourse.tile as tile
from concourse import bass_utils

N_N = 100000
N_E = 1600000
D = 128
N_CORES = 8
NPC = N_N // N_CORES  # nodes per core
P = 128
DG = 256  # destination-group width
GPC = -(-NPC // DG)  # dest groups per core (49)
GH = -(-NPC // P)  # 128-row output tiles per core (98)
NCH = 4  # source chunks (int16 gather indices)
CH = N_N // NCH  # chunk size (25000)
NBC = -(-CH // P)  # row-blocks per chunk (196, last partial)
SBW = 4  # dest groups per superblock
NSB = -(-GPC // SBW)  # superblocks per core (13)
MAXGI = 64  # max 128-row blocks per dma_gather call (multi-packet)

F32 = mybir.dt.float32
F16 = mybir.dt.float16
I16 = mybir.dt.int16

OFF_PAD = 300.0  # is_equal(iota 0..255, 300) is always false


def prep(edges):
    """Integer-only host preprocessing.

    Buckets edges by (dest core, dest group of 128, source chunk of 25000),
    pads each bucket to a shared (max-over-cores) number of 128-edge blocks,
    and packs per-slot metadata:
      - idx16: int16 gather indices, dma_gather layout ([128, slots/16],
        idx i at [i%16, i//16], replicated over the 8 16-partition groups)
      - off32: fp32 destination offset within the group (pad slots: 300)
      - degt:  fp32 integer-valued source degree table (chunk-blocked layout)
      - degh:  fp32 integer-valued dest degree table per core
    """
    hs = np.asarray(edges[0], dtype=np.int64)
    ts = np.asarray(edges[1], dtype=np.int64)
    n_e = hs.shape[0]
    deg = np.bincount(hs, minlength=N_N)

    core = hs // NPC
    local = hs - core * NPC
    g = local // DG
    off = local - g * DG
    ch = ts // CH

    sb = g // SBW
    gw = g - sb * SBW
    bucket = ((core * NSB + sb) * NCH + ch) * SBW + gw
    nbkt = N_CORES * NSB * NCH * SBW

    order = np.lexsort((ts, bucket))
    bkt_s = bucket[order]
    tloc_s = (ts - ch * CH)[order]
    off_s = off[order]

    counts = np.bincount(bkt_s, minlength=nbkt).reshape(N_CORES, NSB, NCH, SBW)
    nblk_sc = -(-counts.max(axis=0) // P)  # [NSB, NCH, SBW]
    nblk = np.zeros((GPC, NCH), np.int64)
    for s in range(NSB):
        for c in range(NCH):
            for w in range(SBW):
                gidx = s * SBW + w
                if gidx < GPC:
                    nblk[gidx, c] = nblk_sc[s, c, w]
    slots_per_bucket = (nblk_sc * P).astype(np.int64)
    tot_slots = int(slots_per_bucket.sum())
    tot_blk = tot_slots // P

    flat_slots = slots_per_bucket.reshape(-1)
    starts1 = np.zeros(NSB * NCH * SBW + 1, np.int64)
    np.cumsum(flat_slots, out=starts1[1:])
    within = bkt_s % (NSB * NCH * SBW)
    counts_flat = np.bincount(bkt_s, minlength=nbkt)
    bstarts = np.zeros(nbkt + 1, np.int64)
    np.cumsum(counts_flat, out=bstarts[1:])
    pos_in_bucket = np.arange(n_e, dtype=np.int64) - bstarts[bkt_s]
    slot = starts1[within] + pos_in_bucket
    core_s = bkt_s // (NSB * NCH * SBW)

    idx_pad = np.zeros((N_CORES, tot_slots), np.int16)
    off_pad = np.full((N_CORES, tot_slots), OFF_PAD, np.float32)
    idx_pad[core_s, slot] = tloc_s.astype(np.int16)
    off_pad[core_s, slot] = off_s.astype(np.float32)

    idx16 = np.ascontiguousarray(
        np.tile(
            idx_pad.reshape(N_CORES, tot_slots // 16, 16).transpose(0, 2, 1),
            (1, 8, 1),
        )
    )
    off32 = np.ascontiguousarray(
        off_pad.reshape(N_CORES, tot_blk, P).transpose(0, 2, 1)
    )

    # source degree table, chunk-blocked: degt[p, c*NBC+b] = deg[c*CH+b*128+p]
    degt = np.ones((P, NCH * NBC), np.float32)
    for c in range(NCH):
        dc = deg[c * CH : (c + 1) * CH].astype(np.float32)
        dc = np.concatenate([dc, np.ones(NBC * P - CH, np.float32)])
        degt[:, c * NBC : (c + 1) * NBC] = dc.reshape(NBC, P).T
    # dest degree table per core: degh[core][p, j] = deg[core*NPC + j*128 + p]
    degh = np.ones((N_CORES, P, GH), np.float32)
    for cr in range(N_CORES):
        dc = deg[cr * NPC : (cr + 1) * NPC].astype(np.float32)
        dc = np.concatenate([dc, np.ones(GH * P - NPC, np.float32)])
        degh[cr] = dc.reshape(GH, P).T

    return nblk, idx16, off32, degt, np.ascontiguousarray(degh)


def build_gcn(nblk, g_bufs=3, s_bufs=3):
    """Build the SPMD Bass program (identical for all cores)."""
    nblk = np.asarray(nblk)
    tot_blk = int(nblk.sum())
    tot_slots = tot_blk * P

    nc = bacc.Bacc(
        "TRN2",
        target_bir_lowering=False,
        debug=False,
        enable_asserts=False,
        num_devices=N_CORES,
        num_swdge_queues=4,
    )
    feats_cd = [
        nc.dram_tensor(f"feats{c}", [CH, D], F32, kind="ExternalInput")
        for c in range(NCH)
    ]
    idx_d = nc.dram_tensor("idx16", [P, tot_slots // 16], I16, kind="ExternalInput")
    off_d = nc.dram_tensor("off32", [P, tot_blk], F32, kind="ExternalInput")
    degt_d = nc.dram_tensor("degt", [P, NCH * NBC], F32, kind="ExternalInput")
    degh_d = nc.dram_tensor("degh", [P, GH], F32, kind="ExternalInput")
    wt_d = nc.dram_tensor("wt", [D, D], F32, kind="ExternalInput")
    iota_d = nc.dram_tensor("iota", [P, DG], F32, kind="ExternalInput")
    y_cd = [
        nc.dram_tensor(f"y{c}", [CH, D], F16, kind="Internal") for c in range(NCH)
    ]
    out_d = nc.dram_tensor("out", [GPC * DG, D], F32, kind="ExternalOutput")

    blkcol = np.zeros((GPC, NCH), np.int64)
    col = 0
    for s in range(NSB):
        for c in range(NCH):
            for w in range(SBW):
                gidx = s * SBW + w
                if gidx < GPC:
                    blkcol[gidx, c] = col
                    col += nblk[gidx, c]
    assert col == tot_blk

    with tile.TileContext(nc) as tc:
        with (
            tc.tile_pool(name="const", bufs=1) as cpool,
            tc.tile_pool(name="prep", bufs=1) as ppool,
            tc.tile_pool(name="ypool", bufs=3) as ypool,
            tc.tile_pool(name="gpool", bufs=g_bufs) as gpool,
            tc.tile_pool(name="spool", bufs=s_bufs) as spool,
            tc.tile_pool(name="mpool", bufs=4) as mpool,
            tc.tile_pool(name="opool", bufs=4) as opool,
            tc.tile_pool(name="psA", bufs=4, space="PSUM") as psA,
            tc.tile_pool(name="psB", bufs=2, space="PSUM") as psB,
        ):
            idx_sb = cpool.tile([P, tot_slots // 16], I16, tag="idx")
            nc.sync.dma_start(idx_sb[:], idx_d[:])
            off_sb = cpool.tile([P, tot_blk], F32, tag="off")
            nc.sync.dma_start(off_sb[:], off_d[:])
            iota_sb = cpool.tile([P, DG], F32, tag="iota")
            nc.sync.dma_start(iota_sb[:], iota_d[:])
            wt_sb = ppool.tile([P, P], F32, tag="wt")
            nc.sync.dma_start(wt_sb[:], wt_d[:])
            degt_sb = ppool.tile([P, NCH * NBC], F32, tag="degt")
            nc.sync.dma_start(degt_sb[:], degt_d[:])
            degh_sb = ppool.tile([P, GH], F32, tag="degh")
            nc.sync.dma_start(degh_sb[:], degh_d[:])

            # s tables: s = deg ** -0.5
            rect_sb = ppool.tile([P, NCH * NBC], F32, tag="rect")
            nc.vector.reciprocal(rect_sb[:], degt_sb[:])
            st_sb = cpool.tile([P, NCH * NBC], F32, tag="st")
            nc.scalar.sqrt(st_sb[:], rect_sb[:])
            rech_sb = ppool.tile([P, GH], F32, tag="rech")
            nc.vector.reciprocal(rech_sb[:], degh_sb[:])
            sh_sb = cpool.tile([P, GH], F32, tag="sh")
            nc.scalar.sqrt(sh_sb[:], rech_sb[:])
            wt16_sb = cpool.tile([P, P], F16, tag="wt16")
            nc.scalar.activation(
                wt16_sb[:], wt_sb[:], mybir.ActivationFunctionType.Copy
            )

            # ---- y-pass: y[t] = s[t] * x[t], fp16, into internal DRAM ----
            YB = 16  # row-blocks per piece
            for c in range(NCH):
                full = CH // P  # 195 full blocks
                tail = CH - full * P  # 40 rows
                b0 = 0
                while b0 < full:
                    b1 = min(b0 + YB, full)
                    nb = b1 - b0
                    r0, r1 = b0 * P, b1 * P
                    xin = ypool.tile([P, nb, P], F32, tag="yx", name="yx")
                    nc.sync.dma_start(
                        xin[:],
                        feats_cd[c][r0:r1, :].rearrange("(b p) f -> p b f", p=P),
                    )
                    yout = ypool.tile([P, nb, P], F16, tag="yy", name="yy")
                    stb = (
                        st_sb[:, c * NBC + b0 : c * NBC + b1]
                        .unsqueeze(2)
                        .broadcast_to([P, nb, P])
                    )
                    nc.vector.tensor_tensor(
                        yout[:], xin[:], stb, mybir.AluOpType.mult
                    )
                    nc.sync.dma_start(
                        y_cd[c][r0:r1, :].rearrange("(b p) f -> p b f", p=P),
                        yout[:],
                    )
                    b0 = b1
                # tail rows (partial partition dim)
                xin_t = ypool.tile([tail, 1, P], F32, tag="yxt", name="yxt")
                nc.sync.dma_start(
                    xin_t[:],
                    feats_cd[c][full * P :, :].rearrange(
                        "(b p) f -> p b f", p=tail
                    ),
                )
                yout_t = ypool.tile([tail, 1, P], F16, tag="yyt", name="yyt")
                stb_t = (
                    st_sb[:tail, c * NBC + full : c * NBC + full + 1]
                    .unsqueeze(2)
                    .broadcast_to([tail, 1, P])
                )
                nc.vector.tensor_tensor(
                    yout_t[:], xin_t[:], stb_t, mybir.AluOpType.mult
                )
                nc.sync.dma_start(
                    y_cd[c][full * P :, :].rearrange("(b p) f -> p b f", p=tail),
                    yout_t[:],
                )

            # ---- main loop ----
            gqn = [0]  # round-robin gather queue counter
            for s in range(NSB):
                glist = [s * SBW + w for w in range(SBW) if s * SBW + w < GPC]
                # pack 2 groups' aggregators into one 512-wide PSUM bank
                nq = -(-len(glist) // 2)
                qtiles = []
                for _ in range(nq):
                    qt = psA.tile([P, 2 * DG], F32, tag="agg", name="agg")
                    nc.vector.memset(qt[:], 0.0)
                    qtiles.append(qt)
                aggs = {}
                done = {}
                for i, g in enumerate(glist):
                    if int(nblk[g].sum()) > 0:
                        q, r = divmod(i, 2)
                        aggs[g] = qtiles[q][:, r * DG : (r + 1) * DG]
                        done[g] = 0
                for c in range(NCH):
                    nb = int(sum(nblk[g, c] for g in glist))
                    if nb == 0:
                        continue
                    col0 = int(blkcol[glist[0], c])
                    gtile = gpool.tile([P, nb, P], F16, tag="G")
                    # split into <=512-row single-packet pieces spread
                    # round-robin over the 4 SWDGE queues: queues parallelize
                    # Q7 descriptor emission, single-packet mode packs 16
                    # rows/packet so the SDMA drain runs ~2x faster
                    b0 = 0
                    while b0 < nb:
                        b1 = min(b0 + 4, nb)
                        slot0 = (col0 + b0) * P
                        slot1 = (col0 + b1) * P
                        nc.gpsimd.dma_gather(
                            gtile[:, b0:b1, :],
                            y_cd[c][:],
                            idx_sb[:, slot0 // 16 : slot1 // 16],
                            (b1 - b0) * P,
                            (b1 - b0) * P,
                            D,
                            single_packet=True,
                            queue_num=gqn[0] % 4,
                        )
                        gqn[0] += 1
                        b0 = b1
                    stile = spool.tile([P, nb, DG], F16, tag="S")
                    iota_bc = iota_sb[:].unsqueeze(1).broadcast_to([P, nb, DG])
                    off_bc = (
                        off_sb[:, col0 : col0 + nb]
                        .unsqueeze(2)
                        .broadcast_to([P, nb, DG])
                    )
                    nc.vector.tensor_tensor(
                        stile[:], iota_bc, off_bc, mybir.AluOpType.is_equal
                    )
                    for g in glist:
                        tot_g = int(nblk[g].sum())
                        for k in range(int(nblk[g, c])):
                            col = int(blkcol[g, c]) + k
                            nc.tensor.matmul(
                                aggs[g][:],
                                lhsT=gtile[:, col - col0, :],
                                rhs=stile[:, col - col0, :],
                                start=False,
                                stop=(done[g] == tot_g - 1),
                                skip_group_check=True,
                            )
                            done[g] += 1
                # linear + scaled relu per group
                for g in glist:
                    if g not in aggs:
                        continue
                    msgt = mpool.tile([P, DG], F16, tag="msgt")
                    nc.scalar.activation(
                        msgt[:], aggs[g][:], mybir.ActivationFunctionType.Copy
                    )
                    for j in range(DG // P):
                        out2 = psB.tile([P, P], F32, tag="out2")
                        nc.tensor.matmul(
                            out2[:],
                            lhsT=msgt[:, j * P : (j + 1) * P],
                            rhs=wt16_sb[:],
                            start=True,
                            stop=True,
                        )
                        osb = opool.tile([P, P], F32, tag="osb")
                        jj = g * (DG // P) + j
                        nc.scalar.activation(
                            osb[:],
                            out2[:],
                            mybir.ActivationFunctionType.Relu,
                            scale=sh_sb[:, jj : jj + 1],
                        )
                        nc.sync.dma_start(
                            out_d[g * DG + j * P : g * DG + (j + 1) * P, :],
                            osb[:],
                        )

    nc.compile()
    return nc


_CACHE = {}


def _run(feats_n, edges, weight, trace=False):
    feats = np.ascontiguousarray(np.asarray(feats_n, dtype=np.float32))
    weight = np.asarray(weight, dtype=np.float32)
    nblk, idx16, off32, degt, degh = prep(edges)

    key = nblk.tobytes()
    if key not in _CACHE:
        _CACHE[key] = build_gcn(nblk)
    nc = _CACHE[key]

    wt = np.ascontiguousarray(weight.T)
    iota = np.ascontiguousarray(
        np.broadcast_to(np.arange(DG, dtype=np.float32), (P, DG))
    )
    in_maps = [
        {
            **{
                f"feats{j}": np.ascontiguousarray(feats[j * CH : (j + 1) * CH])
                for j in range(NCH)
            },
            "idx16": idx16[c],
            "off32": off32[c],
            "degt": degt,
            "degh": degh[c],
            "wt": wt,
            "iota": iota,
        }
        for c in range(N_CORES)
    ]
    res = bass_utils.run_bass_kernel_spmd(
        nc, in_maps, core_ids=list(range(N_CORES)), trace=trace
    )
    out = np.concatenate(
        [res.results[c]["out"][:NPC] for c in range(N_CORES)], axis=0
    )
    return np.ascontiguousarray(out, dtype=np.float32), res


def kernel(feats_n, edges, weight):
    out, _ = _run(feats_n, edges, weight)
    return out
